# revision 16
# baseline (speedup 1.0000x reference)
"""Trainium2 Bass kernel for a Difusco-style GNN backbone (12-layer edge/node
message passing, D=256, N=10000, E=320000), sharded over 8 NeuronCores.

Sharding: edges are partitioned by source-node range. Node ids are padded to
N_PAD = n_cores x blocks_per_core x 128. Core c owns node rows
[npc*c, npc*(c+1)) and all edges whose src falls in that range, grouped by
128-node block and padded per block to a fixed capacity C. The per-layer
segment_sum then becomes, per 128-edge tile, a one-hot matmul accumulated
into the local block accumulator; cores exchange updated node features with
one AllGather per layer.

Compute dtype: bf16 matmul inputs with fp32 PSUM accumulation; the edge/node
residual streams, layernorm statistics and gating stay fp32. h[dst]/h[src]
row gathers use the batched SWDGE dma_gather (512 rows per instruction) from
per-layer bf16 DRAM tables.
"""

import sys
from dataclasses import dataclass

for _p in ("/opt/trn_rl_repo", "/root/.axon_site/_ro/trn_rl_repo"):
    if _p not in sys.path:
        sys.path.insert(0, _p)

import ml_dtypes
import numpy as np

import concourse.bass as bass
import concourse.bacc as bacc
import concourse.tile as tile
import concourse.mybir as mybir
from concourse import bass_utils
from concourse.bass import ds
from concourse.masks import make_identity

F32 = mybir.dt.float32
BF16 = mybir.dt.bfloat16
I32 = mybir.dt.int32
I16 = mybir.dt.int16
AF = mybir.ActivationFunctionType
ALU = mybir.AluOpType
BF = ml_dtypes.bfloat16

D = 256
EPS = 1e-5
SUPER = 512                        # edges per DMA super-tile


@dataclass(frozen=True)
class Cfg:
    n_cores: int = 8
    blocks_per_core: int = 10
    L: int = 12
    n_nodes: int = 10000
    n_edges: int = 320000
    C: int = 4608                  # per-block edge capacity (mult of SUPER)

    @property
    def n_pad(self):
        return self.n_cores * self.blocks_per_core * 128

    @property
    def npc(self):                 # nodes per core
        return self.blocks_per_core * 128

    @property
    def S_e(self):                 # edge slots per core
        return self.blocks_per_core * self.C

    @property
    def TPB(self):                 # 128-edge tiles per block
        return self.C // 128


# --------------------------------------------------------------------------
# host-side math
# --------------------------------------------------------------------------

def _sinusoidal(values, dim):
    half = dim // 2
    freqs = np.exp(-np.log(np.float32(10000.0)) *
                   np.arange(half, dtype=np.float32) / np.float32(half))
    a = values[..., None].astype(np.float32) * freqs
    return np.concatenate([np.sin(a), np.cos(a)], axis=-1).astype(np.float32)


def _silu(x):
    return x / (1.0 + np.exp(-x))


def _wrap16(idx, S_e):
    """[S_e] indices -> [128, S_e//16] int16 in dma_gather's wrapped layout:
    within each 512-index group, index i lives at [i % 16, base + i // 16]."""
    g = idx.reshape(S_e // SUPER, SUPER // 16, 16)        # [G, 32, 16]
    w = np.swapaxes(g, 1, 2)                              # [G, 16, 32]
    rows16 = np.concatenate(list(w), axis=-1).astype(np.int16)
    # replicate across the 8 Q7 cores (each owns 16 partitions)
    out = np.tile(rows16, (8, 1))
    return np.ascontiguousarray(out)


def host_prepare(node_coords, edge_index, edge_distances, x_t, t, params,
                 base_cfg=None):
    """Compute embeddings, folded weights and the per-core edge layout."""
    base_cfg = base_cfg or Cfg()
    src = np.asarray(edge_index[0]).astype(np.int64)
    dst = np.asarray(edge_index[1]).astype(np.int64)
    n_nodes = node_coords.shape[0]
    n_edges = src.shape[0]
    L = np.asarray(params["P"]).shape[0]

    # ---- edge layout -----------------------------------------------------
    gblk = src // 128
    n_blocks_tot = base_cfg.n_cores * base_cfg.blocks_per_core
    counts = np.bincount(gblk, minlength=n_blocks_tot)
    C = int(np.ceil(max(int(counts.max()), 1) / SUPER) * SUPER)
    cfg = Cfg(n_cores=base_cfg.n_cores,
              blocks_per_core=base_cfg.blocks_per_core,
              L=L, n_nodes=n_nodes, n_edges=n_edges, C=C)

    h0 = np.concatenate([_sinusoidal(node_coords[:, 0], D // 2),
                         _sinusoidal(node_coords[:, 1], D // 2)], axis=-1)
    h0 = np.concatenate(
        [h0, np.zeros((cfg.n_pad - n_nodes, D), np.float32)], axis=0)

    e0 = np.concatenate([_sinusoidal(np.asarray(edge_distances), D // 2),
                         _sinusoidal(np.asarray(x_t), D // 2)], axis=-1)

    # time MLP (tiny) on host, folded into the per-layer eb2 bias
    te = _sinusoidal(np.asarray(t).astype(np.float32), D)
    te = _silu(te @ params["time_proj_w1"] + params["time_proj_b1"])
    te = te @ params["time_proj_w2"] + params["time_proj_b2"]     # (1, D)

    order = np.argsort(gblk, kind="stable")
    sorted_gblk = gblk[order]
    block_start = np.searchsorted(sorted_gblk, np.arange(n_blocks_tot))
    rank = np.arange(n_edges) - block_start[sorted_gblk]
    core_of = sorted_gblk // cfg.blocks_per_core
    slot = (sorted_gblk % cfg.blocks_per_core) * C + rank

    per_core = []
    for c in range(cfg.n_cores):
        m = core_of == c
        sl = slot[m]
        ids = order[m]
        srcc = src[ids]
        dstc = dst[ids]
        S_e = cfg.S_e

        e0T = np.zeros((S_e, D), np.float32)
        e0T[sl] = e0[ids]
        e0T = np.ascontiguousarray(e0T.T.reshape(2, 128, S_e))

        srel = np.full(S_e, 999.0, np.float32)
        srel[sl] = (srcc % 128).astype(np.float32)
        srel = np.ascontiguousarray(srel.reshape(S_e // 128, 128).T)

        sgat = np.zeros(S_e, np.int64)
        sgat[sl] = srcc - c * cfg.npc
        didx = np.zeros(S_e, np.int64)
        didx[sl] = dstc

        per_core.append(dict(e0T=e0T, srel=srel,
                             gq16=_wrap16(sgat, S_e), gd16=_wrap16(didx, S_e),
                             slot=sl, ids=ids))

    # ---- weights (folded / pre-arranged) ---------------------------------
    def pmajor(w):      # [.., din, dout] -> [.., 128, 2, dout] p-major chunks
        shp = w.shape
        return np.ascontiguousarray(
            w.reshape(*shp[:-2], 2, 128, shp[-1]).swapaxes(-3, -2)
            .astype(BF))

    P_ = np.asarray(params["P"], np.float32)
    en_s = np.asarray(params["en_s"], np.float32)
    en_b = np.asarray(params["en_b"], np.float32)
    ew1 = np.asarray(params["ew1"], np.float32)
    ew2 = np.asarray(params["ew2"], np.float32)

    mlp_t = np.stack([
        np.maximum(te @ np.asarray(params["tw1"][l], np.float32)
                   + params["tb1"][l], 0.0)
        @ np.asarray(params["tw2"][l], np.float32) + params["tb2"][l]
        for l in range(L)], axis=0).reshape(L, D).astype(np.float32)

    W1f = en_s[:, :, None] * ew1
    b1f = np.asarray(params["eb1"], np.float32) + \
        np.einsum("ld,ldo->lo", en_b, ew1)
    b2f = np.asarray(params["eb2"], np.float32) + mlp_t

    RV = np.concatenate([np.asarray(params["R"], np.float32),
                         np.asarray(params["V"], np.float32)], axis=-1)

    weights = dict(
        Pw=pmajor(P_),
        Qw=pmajor(np.asarray(params["Q"], np.float32)),
        Uw=pmajor(np.asarray(params["U"], np.float32)),
        RVw=pmajor(RV),
        W1=pmajor(W1f.astype(np.float32)), W2=pmajor(ew2),
        b1=np.ascontiguousarray(b1f.reshape(L, 2, 128).swapaxes(1, 2)),
        b2=np.ascontiguousarray(b2f.reshape(L, 2, 128).swapaxes(1, 2)),
        nns=np.ascontiguousarray(
            np.broadcast_to(np.asarray(params["nn_s"], np.float32)[:, None, :],
                            (L, 128, D))),
        nnb=np.ascontiguousarray(
            np.broadcast_to(np.asarray(params["nn_b"], np.float32)[:, None, :],
                            (L, 128, D))),
        HW1=pmajor(np.asarray(params["head_w1"], np.float32)),
        hb1=np.ascontiguousarray(
            np.asarray(params["head_b1"], np.float32).reshape(2, 128).T),
        HW2=np.ascontiguousarray(
            np.asarray(params["head_w2"], np.float32).reshape(2, 128, 2)
            .swapaxes(0, 1).astype(BF)),
        iota=np.broadcast_to(np.arange(128, dtype=np.float32)[None, :],
                             (128, 128)).copy(),
    )
    return h0, per_core, weights, cfg


# --------------------------------------------------------------------------
# device program
# --------------------------------------------------------------------------

def build_program(cfg: Cfg, parts=("node", "local", "edge", "nodeupd",
                                   "coll", "head"), edge_level=6,
                  debug_dump=False):
    L, C, S_e, TPB = cfg.L, cfg.C, cfg.S_e, cfg.TPB
    BPC = cfg.blocks_per_core
    N_PAD = cfg.n_pad
    SPB = C // SUPER               # super-tiles per block

    nc = bacc.Bacc("TRN2", target_bir_lowering=False, debug=False,
                   num_devices=cfg.n_cores)

    e0T_d = nc.dram_tensor("e0T", [2, 128, S_e], F32, kind="ExternalInput")
    h0_d = nc.dram_tensor("h0", [N_PAD, D], F32, kind="ExternalInput")
    srel_d = nc.dram_tensor("srel", [128, S_e // 128], F32, kind="ExternalInput")
    gq16_d = nc.dram_tensor("gq16", [128, S_e // 16], I16, kind="ExternalInput")
    gd16_d = nc.dram_tensor("gd16", [128, S_e // 16], I16, kind="ExternalInput")

    Pw_d = nc.dram_tensor("Pw", [L, 128, 2, D], BF16, kind="ExternalInput")
    Qw_d = nc.dram_tensor("Qw", [L, 128, 2, D], BF16, kind="ExternalInput")
    Uw_d = nc.dram_tensor("Uw", [L, 128, 2, D], BF16, kind="ExternalInput")
    RVw_d = nc.dram_tensor("RVw", [L, 128, 2, 2 * D], BF16, kind="ExternalInput")
    W1_d = nc.dram_tensor("W1", [L, 128, 2, D], BF16, kind="ExternalInput")
    W2_d = nc.dram_tensor("W2", [L, 128, 2, D], BF16, kind="ExternalInput")
    b1_d = nc.dram_tensor("b1", [L, 128, 2], F32, kind="ExternalInput")
    b2_d = nc.dram_tensor("b2", [L, 128, 2], F32, kind="ExternalInput")
    nns_d = nc.dram_tensor("nns", [L, 128, D], F32, kind="ExternalInput")
    nnb_d = nc.dram_tensor("nnb", [L, 128, D], F32, kind="ExternalInput")
    HW1_d = nc.dram_tensor("HW1", [128, 2, D], BF16, kind="ExternalInput")
    hb1_d = nc.dram_tensor("hb1", [128, 2], F32, kind="ExternalInput")
    HW2_d = nc.dram_tensor("HW2", [128, 2, 2], BF16, kind="ExternalInput")
    iota_d = nc.dram_tensor("iota", [128, 128], F32, kind="ExternalInput")

    outT_d = nc.dram_tensor("outT", [2, S_e], F32, kind="ExternalOutput")
    if debug_dump:
        edump_d = nc.dram_tensor("edump", [2, 128, S_e], F32,
                                 kind="ExternalOutput")
        hdump_d = nc.dram_tensor("hdump", [N_PAD, D], F32,
                                 kind="ExternalOutput")
        adump_d = nc.dram_tensor("adump", [128, BPC * D], F32,
                                 kind="ExternalOutput")

    with tile.TileContext(nc) as tc:
        with tc.tile_pool(name="const", bufs=1) as cpool, \
             tc.tile_pool(name="wpool", bufs=2) as wpool, \
             tc.tile_pool(name="sbuf", bufs=1) as sb, \
             tc.tile_pool(name="psum", bufs=1, space="PSUM") as ps, \
             tc.tile_pool(name="dram", bufs=1, space="DRAM") as dr:

            pid = nc.partition_id()
            core_base = pid * cfg.npc

            iota_sb = cpool.tile([128, 128], F32)
            nc.sync.dma_start(out=iota_sb[:], in_=iota_d[:])
            ident = cpool.tile([128, 128], F32)
            make_identity(nc, ident[:])
            identb = cpool.tile([128, 128], BF16)
            nc.vector.tensor_copy(identb[:], ident[:])
            hw1_sb = cpool.tile([128, 2, D], BF16)
            nc.sync.dma_start(out=hw1_sb[:], in_=HW1_d[:])
            hb1_sb = cpool.tile([128, 2], F32)
            nc.sync.dma_start(out=hb1_sb[:], in_=hb1_d[:])
            hw2_sb = cpool.tile([128, 2, 2], BF16)
            nc.sync.dma_start(out=hw2_sb[:], in_=HW2_d[:])
            eps_sb = cpool.tile([128, 1], F32)
            nc.vector.memset(eps_sb[:], EPS)

            e_prev = e0T_d
            h_prev = h0_d

            hu_sb = [sb.tile([128, D], F32, name=f"hu{b}", tag=f"hu{b}")
                     for b in range(BPC)]

            def ln_scale_bias(s1, s2, pfx, n, bufs=4):
                """[128,n] sum/sumsq -> (rstd, -mu*rstd) [128,n] tiles."""
                negmu = sb.tile([128, n], F32, name=f"{pfx}negmu",
                                tag=f"{pfx}negmu", bufs=bufs)
                nc.vector.tensor_scalar_mul(negmu[:], s1[:], -1.0 / D)
                mu2 = sb.tile([128, n], F32, name=f"{pfx}mu2",
                              tag=f"{pfx}mu2", bufs=bufs)
                nc.vector.tensor_tensor(out=mu2[:], in0=negmu[:],
                                        in1=negmu[:], op=ALU.mult)
                va = sb.tile([128, n], F32, name=f"{pfx}va", tag=f"{pfx}va",
                             bufs=bufs)
                nc.vector.scalar_tensor_tensor(
                    out=va[:], in0=s2[:], scalar=1.0 / D, in1=mu2[:],
                    op0=ALU.mult, op1=ALU.subtract)
                lnv = sb.tile([128, n], F32, name=f"{pfx}lnv", tag=f"{pfx}lnv",
                              bufs=bufs)
                nc.scalar.activation(lnv[:], va[:], AF.Ln,
                                     bias=eps_sb[:, 0:1])
                rstd = sb.tile([128, n], F32, name=f"{pfx}rstd",
                               tag=f"{pfx}rstd", bufs=bufs)
                nc.scalar.activation(rstd[:], lnv[:], AF.Exp, scale=-0.5)
                nmk = sb.tile([128, n], F32, name=f"{pfx}nmk", tag=f"{pfx}nmk",
                              bufs=bufs)
                nc.vector.tensor_tensor(out=nmk[:], in0=negmu[:], in1=rstd[:],
                                        op=ALU.mult)
                return rstd, nmk

            for layer in range(L):
                Pw = wpool.tile([128, 2, D], BF16, tag="Pw")
                nc.sync.dma_start(out=Pw[:], in_=Pw_d[layer])
                Qw = wpool.tile([128, 2, D], BF16, tag="Qw")
                nc.sync.dma_start(out=Qw[:], in_=Qw_d[layer])
                Uw = wpool.tile([128, 2, D], BF16, tag="Uw")
                nc.sync.dma_start(out=Uw[:], in_=Uw_d[layer])
                RVw = wpool.tile([128, 2, 2 * D], BF16, tag="RVw")
                nc.sync.dma_start(out=RVw[:], in_=RVw_d[layer])
                W1 = wpool.tile([128, 2, D], BF16, tag="W1")
                nc.sync.dma_start(out=W1[:], in_=W1_d[layer])
                W2 = wpool.tile([128, 2, D], BF16, tag="W2")
                nc.sync.dma_start(out=W2[:], in_=W2_d[layer])
                b1c = wpool.tile([128, 2], F32, tag="b1c")
                nc.sync.dma_start(out=b1c[:], in_=b1_d[layer])
                b2c = wpool.tile([128, 2], F32, tag="b2c")
                nc.sync.dma_start(out=b2c[:], in_=b2_d[layer])
                nns = wpool.tile([128, D], F32, tag="nns")
                nc.sync.dma_start(out=nns[:], in_=nns_d[layer])
                nnb = wpool.tile([128, D], F32, tag="nnb")
                nc.sync.dma_start(out=nnb[:], in_=nnb_d[layer])

                hrv_table = dr.tile([N_PAD, 2 * D], BF16, tag="hrvt", bufs=2)
                hq_table = dr.tile([cfg.npc, D], BF16, tag="hqt", bufs=2)

                # ---------- node phase: hRV table for all nodes ----------
                def node_body(nrow):
                    h_t = sb.tile([128, D], F32, name="h_t", tag="h_t", bufs=3)
                    nc.sync.dma_start(out=h_t[:], in_=h_prev[ds(nrow, 128), :])
                    tr = ps.tile([128, 2, 128], F32, name="tr", tag="ps_tr")
                    nc.tensor.transpose(tr[:, 0, :], h_t[:, 0:128], ident[:])
                    nc.tensor.transpose(tr[:, 1, :], h_t[:, 128:256], ident[:])
                    hT = sb.tile([128, 2, 128], BF16, name="hT", tag="hT",
                                 bufs=3)
                    nc.scalar.copy(hT[:], tr[:])
                    rvp = ps.tile([128, 2 * D], F32, name="rvp", tag="ps_rv")
                    nc.tensor.matmul(rvp[:], lhsT=hT[:, 0, :], rhs=RVw[:, 0, :],
                                     start=True, stop=False)
                    nc.tensor.matmul(rvp[:], lhsT=hT[:, 1, :], rhs=RVw[:, 1, :],
                                     start=False, stop=True)
                    rv_sb = sb.tile([128, 2 * D], BF16, name="rv_sb",
                                    tag="rv_sb", bufs=3)
                    nc.scalar.copy(rv_sb[:], rvp[:])
                    nc.sync.dma_start(out=hrv_table[ds(nrow, 128), :],
                                      in_=rv_sb[:])

                if "node" in parts:
                    tc.For_i_unrolled(0, N_PAD, 128, node_body, max_unroll=10)

                # ---------- node phase: local hQ table + hU ----------
                for b in range(BPC if "local" in parts else 0):
                    nrow = core_base + b * 128
                    h_t = sb.tile([128, D], F32, name="h_tl", tag="h_t", bufs=3)
                    nc.sync.dma_start(out=h_t[:], in_=h_prev[ds(nrow, 128), :])
                    tr = ps.tile([128, 2, 128], F32, name="trl", tag="ps_tr")
                    nc.tensor.transpose(tr[:, 0, :], h_t[:, 0:128], ident[:])
                    nc.tensor.transpose(tr[:, 1, :], h_t[:, 128:256], ident[:])
                    hT = sb.tile([128, 2, 128], BF16, name="hTl", tag="hT",
                                 bufs=3)
                    nc.scalar.copy(hT[:], tr[:])
                    qp = ps.tile([128, D], F32, name="qp", tag="ps_ehat", bufs=2)
                    nc.tensor.matmul(qp[:], lhsT=hT[:, 0, :], rhs=Qw[:, 0, :],
                                     start=True, stop=False)
                    nc.tensor.matmul(qp[:], lhsT=hT[:, 1, :], rhs=Qw[:, 1, :],
                                     start=False, stop=True)
                    hq_sb = sb.tile([128, D], BF16, name="hq_sb", tag="hq_sb",
                                    bufs=2)
                    nc.scalar.copy(hq_sb[:], qp[:])
                    nc.sync.dma_start(out=hq_table[b * 128:(b + 1) * 128, :],
                                      in_=hq_sb[:])
                    up = ps.tile([128, D], F32, name="up", tag="ps_ehat", bufs=2)
                    nc.tensor.matmul(up[:], lhsT=hT[:, 0, :], rhs=Uw[:, 0, :],
                                     start=True, stop=False)
                    nc.tensor.matmul(up[:], lhsT=hT[:, 1, :], rhs=Uw[:, 1, :],
                                     start=False, stop=True)
                    nc.scalar.copy(hu_sb[b][:], up[:])

                # ---------- edge phase ----------
                e_next = dr.tile([2, 128, S_e], F32, tag="ebuf", bufs=2,
                                 name="e_next")
                agg = sb.tile([128, BPC * D], F32, tag="agg")
                nc.gpsimd.memset(agg[:], 0.0)

                with tc.For_i(0, BPC if "edge" in parts else 0, 1,
                              name=f"blk{layer}") as b:
                    sr_sb = sb.tile([128, TPB], F32, name="sr_sb", tag="sr",
                                    bufs=2)
                    nc.sync.dma_start(out=sr_sb[:],
                                      in_=srel_d[:, ds(b * TPB, TPB)])
                    gq_sb = sb.tile([128, C // 16], I16, name="gq_sb", tag="gq",
                                    bufs=2)
                    nc.sync.dma_start(out=gq_sb[:],
                                      in_=gq16_d[:, ds(b * (C // 16), C // 16)])
                    gd_sb = sb.tile([128, C // 16], I16, name="gd_sb", tag="gd",
                                    bufs=2)
                    nc.sync.dma_start(out=gd_sb[:],
                                      in_=gd16_d[:, ds(b * (C // 16), C // 16)])

                    for s in range(SPB):
                        ebase = b * C + s * SUPER
                        isl = slice(s * (SUPER // 16), (s + 1) * (SUPER // 16))
                        eT = sb.tile([128, 2, SUPER], F32, name="eT", tag="eT",
                                     bufs=3)
                        nc.sync.dma_start(out=eT[:, 0, :],
                                          in_=e_prev[0, :, ds(ebase, SUPER)])
                        nc.sync.dma_start(out=eT[:, 1, :],
                                          in_=e_prev[1, :, ds(ebase, SUPER)])
                        ebf = sb.tile([128, 2, SUPER], BF16, name="ebf",
                                      tag="ebf", bufs=3)
                        nc.vector.tensor_copy(ebf[:], eT[:])

                        hqr = sb.tile([128, 4, D], BF16, name="hqr", tag="hqr",
                                      bufs=2)
                        hrvr = sb.tile([128, 4, 2 * D], BF16, name="hrvr",
                                       tag="hrvr", bufs=2)
                        if edge_level >= 2:
                            nc.gpsimd.dma_gather(
                                out_ap=hqr[:], in_ap=hq_table[:],
                                idxs_ap=gq_sb[:, isl], num_idxs=SUPER,
                                num_idxs_reg=SUPER, elem_size=D)
                            nc.gpsimd.dma_gather(
                                out_ap=hrvr[:], in_ap=hrv_table[:],
                                idxs_ap=gd_sb[:, isl], num_idxs=SUPER,
                                num_idxs_reg=SUPER, elem_size=2 * D)

                        enew = sb.tile([128, 2, SUPER], F32, name="enew",
                                       tag="enew", bufs=2)
                        if edge_level < 3:
                            nc.vector.tensor_copy(enew[:], eT[:])

                        ehat4 = sb.tile([128, 4, D], F32, name="ehat4",
                                        tag="ehat4", bufs=2)
                        s1_4 = sb.tile([128, 4], F32, name="s1_4", tag="s1_4",
                                       bufs=3)
                        s2_4 = sb.tile([128, 4], F32, name="s2_4", tag="s2_4",
                                       bufs=3)
                        scp = ps.tile([128, D], F32, name="scp", tag="ps_sc",
                                      bufs=2)

                        for t in range(4 if edge_level >= 3 else 0):
                            esl = slice(t * 128, (t + 1) * 128)
                            ehp = ps.tile([128, D], F32, name="ehp",
                                          tag="ps_ehat", bufs=2)
                            nc.tensor.matmul(ehp[:], lhsT=ebf[:, 0, esl],
                                             rhs=Pw[:, 0, :],
                                             start=True, stop=False)
                            nc.tensor.matmul(ehp[:], lhsT=ebf[:, 1, esl],
                                             rhs=Pw[:, 1, :],
                                             start=False, stop=True)
                            nc.vector.tensor_tensor(
                                out=ehat4[:, t, :], in0=ehp[:],
                                in1=hqr[:, t, :], op=ALU.add)
                            nc.vector.scalar_tensor_tensor(
                                out=ehat4[:, t, :], in0=hrvr[:, t, 0:D],
                                scalar=0.0, in1=ehat4[:, t, :],
                                op0=ALU.bypass, op1=ALU.add,
                                accum_out=s1_4[:, t:t + 1])
                            sq = sb.tile([128, D], F32, name="sq", tag="sq",
                                         bufs=2)
                            nc.scalar.activation(sq[:], ehat4[:, t, :],
                                                 AF.Square,
                                                 accum_out=s2_4[:, t:t + 1])

                        if edge_level >= 4:
                            rstd4, nmk4 = ln_scale_bias(s1_4, s2_4, "e", 4)
                        for t in range(4 if edge_level >= 4 else 0):
                            esl = slice(t * 128, (t + 1) * 128)
                            en = sb.tile([128, D], BF16, name="en", tag="en",
                                         bufs=2)
                            nc.scalar.activation(en[:], ehat4[:, t, :],
                                                 AF.Identity,
                                                 bias=nmk4[:, t:t + 1],
                                                 scale=rstd4[:, t:t + 1])
                            etr = ps.tile([128, 2, 128], BF16, name="etr",
                                          tag="ps_tr")
                            nc.tensor.transpose(etr[:, 0, :], en[:, 0:128],
                                                identb[:])
                            nc.tensor.transpose(etr[:, 1, :], en[:, 128:256],
                                                identb[:])
                            enT = sb.tile([128, 2, 128], BF16, name="enT",
                                          tag="enT", bufs=2)
                            nc.scalar.copy(enT[:], etr[:])
                            h1p = ps.tile([128, 2, 128], F32, name="h1p",
                                          tag="ps_h1")
                            for j in range(2):
                                jsl = slice(j * 128, (j + 1) * 128)
                                nc.tensor.matmul(h1p[:, j, :],
                                                 lhsT=W1[:, 0, jsl],
                                                 rhs=enT[:, 0, :],
                                                 start=True, stop=False)
                                nc.tensor.matmul(h1p[:, j, :],
                                                 lhsT=W1[:, 1, jsl],
                                                 rhs=enT[:, 1, :],
                                                 start=False, stop=True)
                            h1T = sb.tile([128, 2, 128], BF16, name="h1T",
                                          tag="h1T", bufs=2)
                            for j in range(2):
                                nc.scalar.activation(h1T[:, j, :], h1p[:, j, :],
                                                     AF.Relu,
                                                     bias=b1c[:, j:j + 1])
                            mlp = ps.tile([128, 2, 128], F32, name="mlp",
                                          tag="ps_mlp")
                            for j in range(2):
                                jsl = slice(j * 128, (j + 1) * 128)
                                nc.tensor.matmul(mlp[:, j, :],
                                                 lhsT=W2[:, 0, jsl],
                                                 rhs=h1T[:, 0, :],
                                                 start=True, stop=False)
                                nc.tensor.matmul(mlp[:, j, :],
                                                 lhsT=W2[:, 1, jsl],
                                                 rhs=h1T[:, 1, :],
                                                 start=False, stop=True)
                            for j in range(2):
                                nc.vector.scalar_tensor_tensor(
                                    out=enew[:, j, esl], in0=mlp[:, j, :],
                                    scalar=b2c[:, j:j + 1], in1=eT[:, j, esl],
                                    op0=ALU.add, op1=ALU.add)

                        if edge_level >= 5:
                            expx = sb.tile([128, 4, D], F32, name="expx",
                                           tag="expx", bufs=2)
                            nc.scalar.activation(expx[:], ehat4[:], AF.Exp,
                                                 scale=-1.0)
                            nc.vector.tensor_scalar(
                                out=expx[:], in0=expx[:], scalar1=1.0,
                                scalar2=1e30, op0=ALU.add, op1=ALU.min)
                            grec = sb.tile([128, 4, D], F32, name="grec",
                                           tag="grec", bufs=2)
                            nc.vector.reciprocal_approx_fast(out=grec[:],
                                                             in_=expx[:])
                            msg4 = sb.tile([128, 4, D], BF16, name="msg4",
                                           tag="msg4", bufs=2)
                            nc.vector.tensor_tensor(
                                out=msg4[:], in0=grec[:],
                                in1=hrvr[:, :, D:2 * D], op=ALU.mult)

                        for t in range(4 if edge_level >= 6 else 0):
                            st = s * 4 + t
                            oh = sb.tile([128, 128], BF16, name="oh", tag="oh",
                                         bufs=2)
                            nc.vector.tensor_tensor(
                                out=oh[:],
                                in0=sr_sb[:, st:st + 1].to_broadcast([128, 128]),
                                in1=iota_sb[:], op=ALU.is_equal)
                            nc.tensor.matmul(scp[:], lhsT=oh[:],
                                             rhs=msg4[:, t, :],
                                             start=(t == 0), stop=(t == 3))

                        if edge_level >= 3:
                            nc.sync.dma_start(
                                out=e_next[0, :, ds(ebase, SUPER)],
                                in_=enew[:, 0, :])
                            nc.sync.dma_start(
                                out=e_next[1, :, ds(ebase, SUPER)],
                                in_=enew[:, 1, :])
                        if edge_level >= 6:
                            nc.vector.tensor_tensor(
                                out=agg[:, ds(b * D, D)], in0=scp[:],
                                in1=agg[:, ds(b * D, D)], op=ALU.add)

                # ---------- node update + allgather ----------
                bounce = dr.tile([cfg.npc, D], F32, tag="bounce", bufs=2)
                h_next = dr.tile([N_PAD, D], F32, tag="hnext", bufs=2)
                for b in range(BPC if "nodeupd" in parts else 0):
                    x = sb.tile([128, D], F32, name="x", tag="x", bufs=2)
                    s1 = sb.tile([128, 1], F32, name="ns1", tag="ns1", bufs=4)
                    nc.vector.scalar_tensor_tensor(
                        out=x[:], in0=hu_sb[b][:], scalar=0.0,
                        in1=agg[:, b * D:(b + 1) * D],
                        op0=ALU.bypass, op1=ALU.add, accum_out=s1[:])
                    sq = sb.tile([128, D], F32, name="nsq", tag="sq", bufs=2)
                    s2 = sb.tile([128, 1], F32, name="ns2", tag="ns2", bufs=4)
                    nc.scalar.activation(sq[:], x[:], AF.Square,
                                         accum_out=s2[:])
                    rstd, nmk = ln_scale_bias(s1, s2, "n", 1)
                    z = sb.tile([128, D], F32, name="z", tag="z", bufs=2)
                    nc.scalar.activation(z[:], x[:], AF.Identity, bias=nmk[:],
                                         scale=rstd[:])
                    y = sb.tile([128, D], F32, name="y", tag="y", bufs=2)
                    nc.vector.tensor_tensor(out=y[:], in0=z[:], in1=nns[:],
                                            op=ALU.mult)
                    y2 = sb.tile([128, D], F32, name="y2", tag="y2", bufs=2)
                    nc.vector.tensor_tensor(out=y2[:], in0=y[:], in1=nnb[:],
                                            op=ALU.add)
                    h_t = sb.tile([128, D], F32, name="h_tn", tag="h_t", bufs=3)
                    nc.sync.dma_start(
                        out=h_t[:],
                        in_=h_prev[ds(core_base + b * 128, 128), :])
                    hnew = sb.tile([128, D], F32, name="hnew", tag="hnew",
                                   bufs=2)
                    nc.vector.scalar_tensor_tensor(
                        out=hnew[:], in0=y2[:], scalar=0.0, in1=h_t[:],
                        op0=ALU.max, op1=ALU.add)
                    nc.sync.dma_start(out=bounce[b * 128:(b + 1) * 128, :],
                                      in_=hnew[:])

                if "coll" in parts:
                    nc.gpsimd.collective_compute(
                        "AllGather", ALU.bypass,
                        replica_groups=[list(range(cfg.n_cores))],
                        ins=[bounce[:]], outs=[h_next[:]])
                    h_prev = h_next
                if "edge" in parts:
                    e_prev = e_next

            if debug_dump:
                nc.sync.dma_start(out=edump_d[:], in_=e_prev[:])
                nc.sync.dma_start(out=hdump_d[:], in_=h_prev[:])
                nc.sync.dma_start(out=adump_d[:], in_=agg[:])

            # ---------- head ----------
            def head_body(bb):
                for s in range(SPB):
                    ebase = bb * C + s * SUPER
                    eT = sb.tile([128, 2, SUPER], F32, name="eTh", tag="eT",
                                 bufs=3)
                    nc.sync.dma_start(out=eT[:, 0, :],
                                      in_=e_prev[0, :, ds(ebase, SUPER)])
                    nc.sync.dma_start(out=eT[:, 1, :],
                                      in_=e_prev[1, :, ds(ebase, SUPER)])
                    ebf = sb.tile([128, 2, SUPER], BF16, name="ebfh", tag="ebf",
                                  bufs=3)
                    nc.vector.tensor_copy(ebf[:], eT[:])
                    h1a = ps.tile([128, SUPER], F32, name="h1a", tag="ps_rv")
                    h1b = ps.tile([128, SUPER], F32, name="h1b", tag="ps_sc",
                                  bufs=2)
                    for j, pj in enumerate((h1a, h1b)):
                        jsl = slice(j * 128, (j + 1) * 128)
                        nc.tensor.matmul(pj[:], lhsT=hw1_sb[:, 0, jsl],
                                         rhs=ebf[:, 0, :], start=True,
                                         stop=False)
                        nc.tensor.matmul(pj[:], lhsT=hw1_sb[:, 1, jsl],
                                         rhs=ebf[:, 1, :], start=False,
                                         stop=True)
                    h1s = sb.tile([128, 2, SUPER], BF16, name="h1s", tag="enew",
                                  bufs=2)
                    for j, pj in enumerate((h1a, h1b)):
                        nc.scalar.activation(h1s[:, j, :], pj[:], AF.Relu,
                                             bias=hb1_sb[:, j:j + 1])
                    op = ps.tile([2, SUPER], F32, name="op", tag="ps_tr")
                    nc.tensor.matmul(op[:], lhsT=hw2_sb[:, 0, :],
                                     rhs=h1s[:, 0, :], start=True, stop=False)
                    nc.tensor.matmul(op[:], lhsT=hw2_sb[:, 1, :],
                                     rhs=h1s[:, 1, :], start=False, stop=True)
                    o_sb = sb.tile([2, SUPER], F32, name="o_sb", tag="o_sb",
                                   bufs=3)
                    nc.scalar.copy(o_sb[:], op[:])
                    nc.sync.dma_start(out=outT_d[:, ds(ebase, SUPER)],
                                      in_=o_sb[:])

            with tc.For_i(0, BPC if "head" in parts else 0, 1,
                          name="head") as bb:
                head_body(bb)

    nc.compile()
    return nc


_CACHE = {}


def get_program(cfg: Cfg):
    if cfg not in _CACHE:
        _CACHE[cfg] = build_program(cfg)
    return _CACHE[cfg]


def make_in_maps(per_core, h0, w, cfg):
    in_maps = []
    for c in range(cfg.n_cores):
        pc = per_core[c]
        in_maps.append({
            "e0T": pc["e0T"], "h0": h0,
            "srel": pc["srel"], "gq16": pc["gq16"], "gd16": pc["gd16"],
            "Pw": w["Pw"], "Qw": w["Qw"], "Uw": w["Uw"], "RVw": w["RVw"],
            "W1": w["W1"], "W2": w["W2"], "b1": w["b1"], "b2": w["b2"],
            "nns": w["nns"], "nnb": w["nnb"],
            "HW1": w["HW1"], "hb1": w["hb1"], "HW2": w["HW2"],
            "iota": w["iota"],
        })
    return in_maps


def assemble_output(results, per_core, params, cfg):
    out = np.zeros((cfg.n_edges, 2), np.float32)
    for c in range(cfg.n_cores):
        outT = results[c]["outT"]
        pc = per_core[c]
        out[pc["ids"]] = outT[:, pc["slot"]].T
    return out + np.asarray(params["head_b2"], np.float32)[None, :]


def kernel(node_coords, edge_index, edge_distances, x_t, t, params):
    node_coords = np.asarray(node_coords)
    params = {k: np.asarray(v) for k, v in params.items()}

    h0, per_core, w, cfg = host_prepare(
        node_coords, edge_index, np.asarray(edge_distances), np.asarray(x_t),
        np.asarray(t), params)
    nc = get_program(cfg)
    in_maps = make_in_maps(per_core, h0, w, cfg)
    res = bass_utils.run_bass_kernel_spmd(
        nc, in_maps, core_ids=list(range(cfg.n_cores)))
    return assemble_output(res.results, per_core, params, cfg)


# revision 19
# speedup vs baseline: 3.3372x; 3.3372x over previous
"""Trainium2 Bass kernel for a Difusco-style GNN backbone (12-layer edge/node
message passing, D=256, N=10000, E=320000), sharded over 8 NeuronCores.

Sharding: edges are partitioned by source-node range. Node ids are padded to
N_PAD = n_cores x blocks_per_core x 128. Core c owns node rows
[npc*c, npc*(c+1)) and all edges whose src falls in that range, grouped by
128-node block and padded per block to a fixed capacity C. The per-layer
segment_sum then becomes, per 128-edge tile, a one-hot matmul accumulated
into the local block accumulator; cores exchange updated node features with
one AllGather per layer.

Compute dtype: bf16 matmul inputs with fp32 PSUM accumulation; the edge/node
residual streams, layernorm statistics and gating stay fp32. h[dst]/h[src]
row gathers use the batched SWDGE dma_gather (512 rows per instruction) from
per-layer bf16 DRAM tables.
"""

import sys
from dataclasses import dataclass

for _p in ("/opt/trn_rl_repo", "/root/.axon_site/_ro/trn_rl_repo"):
    if _p not in sys.path:
        sys.path.insert(0, _p)

import ml_dtypes
import numpy as np

import concourse.bass as bass
import concourse.bacc as bacc
import concourse.tile as tile
import concourse.mybir as mybir
from concourse import bass_utils
from concourse.bass import ds
from concourse.masks import make_identity

F32 = mybir.dt.float32
BF16 = mybir.dt.bfloat16
I32 = mybir.dt.int32
I16 = mybir.dt.int16
AF = mybir.ActivationFunctionType
ALU = mybir.AluOpType
BF = ml_dtypes.bfloat16

D = 256
EPS = 1e-5
SUPER = 512                        # edges per DMA super-tile


@dataclass(frozen=True)
class Cfg:
    n_cores: int = 8
    blocks_per_core: int = 10
    L: int = 12
    n_nodes: int = 10000
    n_edges: int = 320000
    C: int = 4608                  # per-block edge capacity (mult of SUPER)

    @property
    def n_pad(self):
        return self.n_cores * self.blocks_per_core * 128

    @property
    def npc(self):                 # nodes per core
        return self.blocks_per_core * 128

    @property
    def S_e(self):                 # edge slots per core
        return self.blocks_per_core * self.C

    @property
    def TPB(self):                 # 128-edge tiles per block
        return self.C // 128


# --------------------------------------------------------------------------
# host-side math
# --------------------------------------------------------------------------

def _sinusoidal(values, dim):
    half = dim // 2
    freqs = np.exp(-np.log(np.float32(10000.0)) *
                   np.arange(half, dtype=np.float32) / np.float32(half))
    a = values[..., None].astype(np.float32) * freqs
    return np.concatenate([np.sin(a), np.cos(a)], axis=-1).astype(np.float32)


def _silu(x):
    return x / (1.0 + np.exp(-x))


def _wrap16(idx, S_e):
    """[S_e] indices -> [128, S_e//16] int16 in dma_gather's wrapped layout:
    within each 512-index group, index i lives at [i % 16, base + i // 16]."""
    g = idx.reshape(S_e // SUPER, SUPER // 16, 16)        # [G, 32, 16]
    w = np.swapaxes(g, 1, 2)                              # [G, 16, 32]
    rows16 = np.concatenate(list(w), axis=-1).astype(np.int16)
    # replicate across the 8 Q7 cores (each owns 16 partitions)
    out = np.tile(rows16, (8, 1))
    return np.ascontiguousarray(out)


def host_prepare(node_coords, edge_index, edge_distances, x_t, t, params,
                 base_cfg=None):
    """Compute embeddings, folded weights and the per-core edge layout."""
    base_cfg = base_cfg or Cfg()
    src = np.asarray(edge_index[0]).astype(np.int64)
    dst = np.asarray(edge_index[1]).astype(np.int64)
    n_nodes = node_coords.shape[0]
    n_edges = src.shape[0]
    L = np.asarray(params["P"]).shape[0]

    # ---- edge layout -----------------------------------------------------
    gblk = src // 128
    n_blocks_tot = base_cfg.n_cores * base_cfg.blocks_per_core
    counts = np.bincount(gblk, minlength=n_blocks_tot)
    C = int(np.ceil(max(int(counts.max()), 1) / SUPER) * SUPER)
    cfg = Cfg(n_cores=base_cfg.n_cores,
              blocks_per_core=base_cfg.blocks_per_core,
              L=L, n_nodes=n_nodes, n_edges=n_edges, C=C)

    h0 = np.concatenate([_sinusoidal(node_coords[:, 0], D // 2),
                         _sinusoidal(node_coords[:, 1], D // 2)], axis=-1)
    h0 = np.concatenate(
        [h0, np.zeros((cfg.n_pad - n_nodes, D), np.float32)], axis=0)

    e0 = np.concatenate([_sinusoidal(np.asarray(edge_distances), D // 2),
                         _sinusoidal(np.asarray(x_t), D // 2)], axis=-1)

    # time MLP (tiny) on host, folded into the per-layer eb2 bias
    te = _sinusoidal(np.asarray(t).astype(np.float32), D)
    te = _silu(te @ params["time_proj_w1"] + params["time_proj_b1"])
    te = te @ params["time_proj_w2"] + params["time_proj_b2"]     # (1, D)

    order = np.argsort(gblk, kind="stable")
    sorted_gblk = gblk[order]
    block_start = np.searchsorted(sorted_gblk, np.arange(n_blocks_tot))
    rank = np.arange(n_edges) - block_start[sorted_gblk]
    core_of = sorted_gblk // cfg.blocks_per_core
    slot = (sorted_gblk % cfg.blocks_per_core) * C + rank

    per_core = []
    for c in range(cfg.n_cores):
        m = core_of == c
        sl = slot[m]
        ids = order[m]
        srcc = src[ids]
        dstc = dst[ids]
        S_e = cfg.S_e

        e0T = np.zeros((S_e, D), np.float32)
        e0T[sl] = e0[ids]
        e0T = np.ascontiguousarray(e0T.T.reshape(2, 128, S_e))

        srel = np.full(S_e, 999.0, np.float32)
        srel[sl] = (srcc % 128).astype(np.float32)
        srel = np.ascontiguousarray(srel.reshape(S_e // 128, 128).T)

        sgat = np.zeros(S_e, np.int64)
        sgat[sl] = srcc - c * cfg.npc
        didx = np.zeros(S_e, np.int64)
        didx[sl] = dstc

        per_core.append(dict(e0T=e0T, srel=srel,
                             gq16=_wrap16(sgat, S_e), gd16=_wrap16(didx, S_e),
                             slot=sl, ids=ids))

    # ---- weights (folded / pre-arranged) ---------------------------------
    def pmajor(w):      # [.., din, dout] -> [.., 128, 2, dout] p-major chunks
        shp = w.shape
        return np.ascontiguousarray(
            w.reshape(*shp[:-2], 2, 128, shp[-1]).swapaxes(-3, -2)
            .astype(BF))

    P_ = np.asarray(params["P"], np.float32)
    en_s = np.asarray(params["en_s"], np.float32)
    en_b = np.asarray(params["en_b"], np.float32)
    ew1 = np.asarray(params["ew1"], np.float32)
    ew2 = np.asarray(params["ew2"], np.float32)

    mlp_t = np.stack([
        np.maximum(te @ np.asarray(params["tw1"][l], np.float32)
                   + params["tb1"][l], 0.0)
        @ np.asarray(params["tw2"][l], np.float32) + params["tb2"][l]
        for l in range(L)], axis=0).reshape(L, D).astype(np.float32)

    W1f = en_s[:, :, None] * ew1
    b1f = np.asarray(params["eb1"], np.float32) + \
        np.einsum("ld,ldo->lo", en_b, ew1)
    b2f = np.asarray(params["eb2"], np.float32) + mlp_t

    RV = np.concatenate([np.asarray(params["R"], np.float32),
                         np.asarray(params["V"], np.float32)], axis=-1)

    weights = dict(
        Pw=pmajor(P_),
        Qw=pmajor(np.asarray(params["Q"], np.float32)),
        Uw=pmajor(np.asarray(params["U"], np.float32)),
        RVw=pmajor(RV),
        W1=pmajor(W1f.astype(np.float32)), W2=pmajor(ew2),
        b1=np.ascontiguousarray(b1f.reshape(L, 2, 128).swapaxes(1, 2)),
        b2=np.ascontiguousarray(b2f.reshape(L, 2, 128).swapaxes(1, 2)),
        nns=np.ascontiguousarray(
            np.broadcast_to(np.asarray(params["nn_s"], np.float32)[:, None, :],
                            (L, 128, D))),
        nnb=np.ascontiguousarray(
            np.broadcast_to(np.asarray(params["nn_b"], np.float32)[:, None, :],
                            (L, 128, D))),
        HW1=pmajor(np.asarray(params["head_w1"], np.float32)),
        hb1=np.ascontiguousarray(
            np.asarray(params["head_b1"], np.float32).reshape(2, 128).T),
        HW2=np.ascontiguousarray(
            np.asarray(params["head_w2"], np.float32).reshape(2, 128, 2)
            .swapaxes(0, 1).astype(BF)),
        iota=np.broadcast_to(np.arange(128, dtype=np.float32)[None, :],
                             (128, 128)).copy(),
    )
    return h0, per_core, weights, cfg


# --------------------------------------------------------------------------
# device program
# --------------------------------------------------------------------------

def build_program(cfg: Cfg, parts=("node", "local", "edge", "nodeupd",
                                   "coll", "head"), edge_level=6,
                  debug_dump=False):
    L, C, S_e, TPB = cfg.L, cfg.C, cfg.S_e, cfg.TPB
    BPC = cfg.blocks_per_core
    N_PAD = cfg.n_pad
    SPB = C // SUPER               # super-tiles per block

    nc = bacc.Bacc("TRN2", target_bir_lowering=False, debug=False,
                   num_devices=cfg.n_cores)

    e0T_d = nc.dram_tensor("e0T", [2, 128, S_e], F32, kind="ExternalInput")
    h0_d = nc.dram_tensor("h0", [N_PAD, D], F32, kind="ExternalInput")
    srel_d = nc.dram_tensor("srel", [128, S_e // 128], F32, kind="ExternalInput")
    gq16_d = nc.dram_tensor("gq16", [128, S_e // 16], I16, kind="ExternalInput")
    gd16_d = nc.dram_tensor("gd16", [128, S_e // 16], I16, kind="ExternalInput")

    Pw_d = nc.dram_tensor("Pw", [L, 128, 2, D], BF16, kind="ExternalInput")
    Qw_d = nc.dram_tensor("Qw", [L, 128, 2, D], BF16, kind="ExternalInput")
    Uw_d = nc.dram_tensor("Uw", [L, 128, 2, D], BF16, kind="ExternalInput")
    RVw_d = nc.dram_tensor("RVw", [L, 128, 2, 2 * D], BF16, kind="ExternalInput")
    W1_d = nc.dram_tensor("W1", [L, 128, 2, D], BF16, kind="ExternalInput")
    W2_d = nc.dram_tensor("W2", [L, 128, 2, D], BF16, kind="ExternalInput")
    b1_d = nc.dram_tensor("b1", [L, 128, 2], F32, kind="ExternalInput")
    b2_d = nc.dram_tensor("b2", [L, 128, 2], F32, kind="ExternalInput")
    nns_d = nc.dram_tensor("nns", [L, 128, D], F32, kind="ExternalInput")
    nnb_d = nc.dram_tensor("nnb", [L, 128, D], F32, kind="ExternalInput")
    HW1_d = nc.dram_tensor("HW1", [128, 2, D], BF16, kind="ExternalInput")
    hb1_d = nc.dram_tensor("hb1", [128, 2], F32, kind="ExternalInput")
    HW2_d = nc.dram_tensor("HW2", [128, 2, 2], BF16, kind="ExternalInput")
    iota_d = nc.dram_tensor("iota", [128, 128], F32, kind="ExternalInput")

    outT_d = nc.dram_tensor("outT", [2, S_e], F32, kind="ExternalOutput")
    if debug_dump:
        edump_d = nc.dram_tensor("edump", [2, 128, S_e], F32,
                                 kind="ExternalOutput")
        hdump_d = nc.dram_tensor("hdump", [N_PAD, D], F32,
                                 kind="ExternalOutput")
        adump_d = nc.dram_tensor("adump", [128, BPC * D], F32,
                                 kind="ExternalOutput")

    with tile.TileContext(nc) as tc:
        with tc.tile_pool(name="const", bufs=1) as cpool, \
             tc.tile_pool(name="wpool", bufs=2) as wpool, \
             tc.tile_pool(name="sbuf", bufs=1) as sb, \
             tc.tile_pool(name="psum", bufs=1, space="PSUM") as ps, \
             tc.tile_pool(name="dram", bufs=1, space="DRAM") as dr:

            pid = nc.partition_id()
            core_base = pid * cfg.npc

            iota_sb = cpool.tile([128, 128], F32)
            nc.sync.dma_start(out=iota_sb[:], in_=iota_d[:])
            ident = cpool.tile([128, 128], F32)
            make_identity(nc, ident[:])
            identb = cpool.tile([128, 128], BF16)
            nc.vector.tensor_copy(identb[:], ident[:])
            hw1_sb = cpool.tile([128, 2, D], BF16)
            nc.sync.dma_start(out=hw1_sb[:], in_=HW1_d[:])
            hb1_sb = cpool.tile([128, 2], F32)
            nc.sync.dma_start(out=hb1_sb[:], in_=hb1_d[:])
            hw2_sb = cpool.tile([128, 2, 2], BF16)
            nc.sync.dma_start(out=hw2_sb[:], in_=HW2_d[:])
            eps_sb = cpool.tile([128, 1], F32)
            nc.vector.memset(eps_sb[:], EPS)

            e_prev = e0T_d
            h_prev = h0_d

            hu_sb = [sb.tile([128, D], F32, name=f"hu{b}", tag=f"hu{b}")
                     for b in range(BPC)]

            def ln_scale_bias(s1, s2, pfx, n, bufs=4):
                """[128,n] sum/sumsq -> (rstd, -mu*rstd) [128,n] tiles."""
                negmu = sb.tile([128, n], F32, name=f"{pfx}negmu",
                                tag=f"{pfx}negmu", bufs=bufs)
                nc.vector.tensor_scalar_mul(negmu[:], s1[:], -1.0 / D)
                mu2 = sb.tile([128, n], F32, name=f"{pfx}mu2",
                              tag=f"{pfx}mu2", bufs=bufs)
                nc.vector.tensor_tensor(out=mu2[:], in0=negmu[:],
                                        in1=negmu[:], op=ALU.mult)
                va = sb.tile([128, n], F32, name=f"{pfx}va", tag=f"{pfx}va",
                             bufs=bufs)
                nc.vector.scalar_tensor_tensor(
                    out=va[:], in0=s2[:], scalar=1.0 / D, in1=mu2[:],
                    op0=ALU.mult, op1=ALU.subtract)
                lnv = sb.tile([128, n], F32, name=f"{pfx}lnv", tag=f"{pfx}lnv",
                              bufs=bufs)
                nc.scalar.activation(lnv[:], va[:], AF.Ln,
                                     bias=eps_sb[:, 0:1])
                rstd = sb.tile([128, n], F32, name=f"{pfx}rstd",
                               tag=f"{pfx}rstd", bufs=bufs)
                nc.scalar.activation(rstd[:], lnv[:], AF.Exp, scale=-0.5)
                nmk = sb.tile([128, n], F32, name=f"{pfx}nmk", tag=f"{pfx}nmk",
                              bufs=bufs)
                nc.vector.tensor_tensor(out=nmk[:], in0=negmu[:], in1=rstd[:],
                                        op=ALU.mult)
                return rstd, nmk

            for layer in range(L):
                Pw = wpool.tile([128, 2, D], BF16, tag="Pw")
                nc.sync.dma_start(out=Pw[:], in_=Pw_d[layer])
                Qw = wpool.tile([128, 2, D], BF16, tag="Qw")
                nc.sync.dma_start(out=Qw[:], in_=Qw_d[layer])
                Uw = wpool.tile([128, 2, D], BF16, tag="Uw")
                nc.sync.dma_start(out=Uw[:], in_=Uw_d[layer])
                RVw = wpool.tile([128, 2, 2 * D], BF16, tag="RVw")
                nc.sync.dma_start(out=RVw[:], in_=RVw_d[layer])
                W1 = wpool.tile([128, 2, D], BF16, tag="W1")
                nc.sync.dma_start(out=W1[:], in_=W1_d[layer])
                W2 = wpool.tile([128, 2, D], BF16, tag="W2")
                nc.sync.dma_start(out=W2[:], in_=W2_d[layer])
                b1c = wpool.tile([128, 2], F32, tag="b1c")
                nc.sync.dma_start(out=b1c[:], in_=b1_d[layer])
                b2c = wpool.tile([128, 2], F32, tag="b2c")
                nc.sync.dma_start(out=b2c[:], in_=b2_d[layer])
                nns = wpool.tile([128, D], F32, tag="nns")
                nc.sync.dma_start(out=nns[:], in_=nns_d[layer])
                nnb = wpool.tile([128, D], F32, tag="nnb")
                nc.sync.dma_start(out=nnb[:], in_=nnb_d[layer])

                hrv_table = dr.tile([N_PAD, 2 * D], BF16, tag="hrvt", bufs=2)
                hq_table = dr.tile([cfg.npc, D], BF16, tag="hqt", bufs=2)

                # ---------- node phase: hRV table for all nodes ----------
                def node_body(nrow):
                    h_t = sb.tile([128, D], F32, name="h_t", tag="h_t", bufs=3)
                    nc.sync.dma_start(out=h_t[:], in_=h_prev[ds(nrow, 128), :])
                    tr = ps.tile([128, 2, 128], F32, name="tr", tag="ps_tr")
                    nc.tensor.transpose(tr[:, 0, :], h_t[:, 0:128], ident[:])
                    nc.tensor.transpose(tr[:, 1, :], h_t[:, 128:256], ident[:])
                    hT = sb.tile([128, 2, 128], BF16, name="hT", tag="hT",
                                 bufs=3)
                    nc.scalar.copy(hT[:], tr[:])
                    rvp = ps.tile([128, 2 * D], F32, name="rvp", tag="ps_rv")
                    nc.tensor.matmul(rvp[:], lhsT=hT[:, 0, :], rhs=RVw[:, 0, :],
                                     start=True, stop=False)
                    nc.tensor.matmul(rvp[:], lhsT=hT[:, 1, :], rhs=RVw[:, 1, :],
                                     start=False, stop=True)
                    rv_sb = sb.tile([128, 2 * D], BF16, name="rv_sb",
                                    tag="rv_sb", bufs=3)
                    nc.scalar.copy(rv_sb[:], rvp[:])
                    nc.sync.dma_start(out=hrv_table[ds(nrow, 128), :],
                                      in_=rv_sb[:])

                if "node" in parts:
                    tc.For_i_unrolled(0, N_PAD, 128, node_body, max_unroll=10)

                # ---------- node phase: local hQ table + hU ----------
                for b in range(BPC if "local" in parts else 0):
                    nrow = core_base + b * 128
                    h_t = sb.tile([128, D], F32, name="h_tl", tag="h_t", bufs=3)
                    nc.sync.dma_start(out=h_t[:], in_=h_prev[ds(nrow, 128), :])
                    tr = ps.tile([128, 2, 128], F32, name="trl", tag="ps_tr")
                    nc.tensor.transpose(tr[:, 0, :], h_t[:, 0:128], ident[:])
                    nc.tensor.transpose(tr[:, 1, :], h_t[:, 128:256], ident[:])
                    hT = sb.tile([128, 2, 128], BF16, name="hTl", tag="hT",
                                 bufs=3)
                    nc.scalar.copy(hT[:], tr[:])
                    qp = ps.tile([128, D], F32, name="qp", tag="ps_ehat", bufs=2)
                    nc.tensor.matmul(qp[:], lhsT=hT[:, 0, :], rhs=Qw[:, 0, :],
                                     start=True, stop=False)
                    nc.tensor.matmul(qp[:], lhsT=hT[:, 1, :], rhs=Qw[:, 1, :],
                                     start=False, stop=True)
                    hq_sb = sb.tile([128, D], BF16, name="hq_sb", tag="hq_sb",
                                    bufs=2)
                    nc.scalar.copy(hq_sb[:], qp[:])
                    nc.sync.dma_start(out=hq_table[b * 128:(b + 1) * 128, :],
                                      in_=hq_sb[:])
                    up = ps.tile([128, D], F32, name="up", tag="ps_ehat", bufs=2)
                    nc.tensor.matmul(up[:], lhsT=hT[:, 0, :], rhs=Uw[:, 0, :],
                                     start=True, stop=False)
                    nc.tensor.matmul(up[:], lhsT=hT[:, 1, :], rhs=Uw[:, 1, :],
                                     start=False, stop=True)
                    nc.scalar.copy(hu_sb[b][:], up[:])

                # ---------- edge phase ----------
                e_next = dr.tile([2, 128, S_e], F32, tag="ebuf", bufs=2,
                                 name="e_next")
                agg = sb.tile([128, BPC * D], F32, tag="agg")
                nc.gpsimd.memset(agg[:], 0.0)

                with tc.For_i(0, BPC if "edge" in parts else 0, 1,
                              name=f"blk{layer}") as b:
                    scp = ps.tile([128, D], F32, name="scp", tag="ps_sc",
                                  bufs=2)
                    sr_sb = sb.tile([128, TPB], F32, name="sr_sb", tag="sr",
                                    bufs=2)
                    nc.sync.dma_start(out=sr_sb[:],
                                      in_=srel_d[:, ds(b * TPB, TPB)])
                    gq_sb = sb.tile([128, C // 16], I16, name="gq_sb", tag="gq",
                                    bufs=2)
                    nc.sync.dma_start(out=gq_sb[:],
                                      in_=gq16_d[:, ds(b * (C // 16), C // 16)])
                    gd_sb = sb.tile([128, C // 16], I16, name="gd_sb", tag="gd",
                                    bufs=2)
                    nc.sync.dma_start(out=gd_sb[:],
                                      in_=gd16_d[:, ds(b * (C // 16), C // 16)])

                    for s in range(SPB):
                        ebase = b * C + s * SUPER
                        isl = slice(s * (SUPER // 16), (s + 1) * (SUPER // 16))
                        eT = sb.tile([128, 2, SUPER], F32, name="eT", tag="eT",
                                     bufs=3)
                        nc.sync.dma_start(out=eT[:, 0, :],
                                          in_=e_prev[0, :, ds(ebase, SUPER)])
                        nc.sync.dma_start(out=eT[:, 1, :],
                                          in_=e_prev[1, :, ds(ebase, SUPER)])
                        ebf = sb.tile([128, 2, SUPER], BF16, name="ebf",
                                      tag="ebf", bufs=3)
                        nc.vector.tensor_copy(ebf[:], eT[:])

                        hqr = sb.tile([128, 4, D], BF16, name="hqr", tag="hqr",
                                      bufs=2)
                        hrvr = sb.tile([128, 4, 2 * D], BF16, name="hrvr",
                                       tag="hrvr", bufs=2)
                        if edge_level >= 2:
                            nc.gpsimd.dma_gather(
                                out_ap=hqr[:], in_ap=hq_table[:],
                                idxs_ap=gq_sb[:, isl], num_idxs=SUPER,
                                num_idxs_reg=SUPER, elem_size=D)
                            nc.gpsimd.dma_gather(
                                out_ap=hrvr[:], in_ap=hrv_table[:],
                                idxs_ap=gd_sb[:, isl], num_idxs=SUPER,
                                num_idxs_reg=SUPER, elem_size=2 * D)

                        enew = sb.tile([128, 2, SUPER], F32, name="enew",
                                       tag="enew", bufs=2)
                        if edge_level < 3:
                            nc.vector.tensor_copy(enew[:], eT[:])

                        ehat4 = sb.tile([128, 4, D], F32, name="ehat4",
                                        tag="ehat4", bufs=2)
                        s1_4 = sb.tile([128, 4], F32, name="s1_4", tag="s1_4",
                                       bufs=3)
                        s2_4 = sb.tile([128, 4], F32, name="s2_4", tag="s2_4",
                                       bufs=3)

                        for t in range(4 if edge_level >= 3 else 0):
                            esl = slice(t * 128, (t + 1) * 128)
                            ehp = ps.tile([128, D], F32, name="ehp",
                                          tag="ps_ehat", bufs=2)
                            nc.tensor.matmul(ehp[:], lhsT=ebf[:, 0, esl],
                                             rhs=Pw[:, 0, :],
                                             start=True, stop=False)
                            nc.tensor.matmul(ehp[:], lhsT=ebf[:, 1, esl],
                                             rhs=Pw[:, 1, :],
                                             start=False, stop=True)
                            nc.vector.tensor_tensor(
                                out=ehat4[:, t, :], in0=ehp[:],
                                in1=hqr[:, t, :], op=ALU.add)
                            nc.vector.scalar_tensor_tensor(
                                out=ehat4[:, t, :], in0=hrvr[:, t, 0:D],
                                scalar=0.0, in1=ehat4[:, t, :],
                                op0=ALU.bypass, op1=ALU.add,
                                accum_out=s1_4[:, t:t + 1])
                            sq = sb.tile([128, D], F32, name="sq", tag="sq",
                                         bufs=2)
                            nc.scalar.activation(sq[:], ehat4[:, t, :],
                                                 AF.Square,
                                                 accum_out=s2_4[:, t:t + 1])

                        if edge_level >= 4:
                            rstd4, nmk4 = ln_scale_bias(s1_4, s2_4, "e", 4)
                        for t in range(4 if edge_level >= 4 else 0):
                            esl = slice(t * 128, (t + 1) * 128)
                            en = sb.tile([128, D], BF16, name="en", tag="en",
                                         bufs=2)
                            nc.scalar.activation(en[:], ehat4[:, t, :],
                                                 AF.Identity,
                                                 bias=nmk4[:, t:t + 1],
                                                 scale=rstd4[:, t:t + 1])
                            etr = ps.tile([128, 2, 128], BF16, name="etr",
                                          tag="ps_tr")
                            nc.tensor.transpose(etr[:, 0, :], en[:, 0:128],
                                                identb[:])
                            nc.tensor.transpose(etr[:, 1, :], en[:, 128:256],
                                                identb[:])
                            enT = sb.tile([128, 2, 128], BF16, name="enT",
                                          tag="enT", bufs=2)
                            nc.scalar.copy(enT[:], etr[:])
                            h1p = ps.tile([128, 2, 128], F32, name="h1p",
                                          tag="ps_h1")
                            for j in range(2):
                                jsl = slice(j * 128, (j + 1) * 128)
                                nc.tensor.matmul(h1p[:, j, :],
                                                 lhsT=W1[:, 0, jsl],
                                                 rhs=enT[:, 0, :],
                                                 start=True, stop=False)
                                nc.tensor.matmul(h1p[:, j, :],
                                                 lhsT=W1[:, 1, jsl],
                                                 rhs=enT[:, 1, :],
                                                 start=False, stop=True)
                            h1T = sb.tile([128, 2, 128], BF16, name="h1T",
                                          tag="h1T", bufs=2)
                            for j in range(2):
                                nc.scalar.activation(h1T[:, j, :], h1p[:, j, :],
                                                     AF.Relu,
                                                     bias=b1c[:, j:j + 1])
                            mlp = ps.tile([128, 2, 128], F32, name="mlp",
                                          tag="ps_mlp")
                            for j in range(2):
                                jsl = slice(j * 128, (j + 1) * 128)
                                nc.tensor.matmul(mlp[:, j, :],
                                                 lhsT=W2[:, 0, jsl],
                                                 rhs=h1T[:, 0, :],
                                                 start=True, stop=False)
                                nc.tensor.matmul(mlp[:, j, :],
                                                 lhsT=W2[:, 1, jsl],
                                                 rhs=h1T[:, 1, :],
                                                 start=False, stop=True)
                            for j in range(2):
                                nc.vector.scalar_tensor_tensor(
                                    out=enew[:, j, esl], in0=mlp[:, j, :],
                                    scalar=b2c[:, j:j + 1], in1=eT[:, j, esl],
                                    op0=ALU.add, op1=ALU.add)

                        if edge_level >= 5:
                            expx = sb.tile([128, 4, D], F32, name="expx",
                                           tag="expx", bufs=2)
                            nc.scalar.activation(expx[:], ehat4[:], AF.Exp,
                                                 scale=-1.0)
                            nc.vector.tensor_scalar(
                                out=expx[:], in0=expx[:], scalar1=1.0,
                                scalar2=1e30, op0=ALU.add, op1=ALU.min)
                            grec = sb.tile([128, 4, D], F32, name="grec",
                                           tag="grec", bufs=2)
                            nc.vector.reciprocal_approx_fast(out=grec[:],
                                                             in_=expx[:])
                            msg4 = sb.tile([128, 4, D], BF16, name="msg4",
                                           tag="msg4", bufs=2)
                            nc.vector.tensor_tensor(
                                out=msg4[:], in0=grec[:],
                                in1=hrvr[:, :, D:2 * D], op=ALU.mult)

                        if edge_level >= 5.5:
                            oh4 = sb.tile([128, 4, 128], BF16, name="oh4",
                                          tag="oh4", bufs=2)
                            st = s * 4
                            sr4 = sr_sb[:, st:st + 4] \
                                .rearrange("p (t o) -> p t o", o=1) \
                                .to_broadcast([128, 4, 128])
                            io4 = iota_sb[:].rearrange("p (o j) -> p o j", o=1) \
                                .to_broadcast([128, 4, 128])
                            nc.vector.tensor_tensor(
                                out=oh4[:], in0=sr4, in1=io4, op=ALU.is_equal)
                            if edge_level >= 5.8:
                                for t in range(4):
                                    nc.tensor.matmul(
                                        scp[:], lhsT=oh4[:, t, :],
                                        rhs=msg4[:, t, :],
                                        start=(s == 0 and t == 0),
                                        stop=(s == SPB - 1 and t == 3))

                        if edge_level >= 3:
                            nc.sync.dma_start(
                                out=e_next[0, :, ds(ebase, SUPER)],
                                in_=enew[:, 0, :])
                            nc.sync.dma_start(
                                out=e_next[1, :, ds(ebase, SUPER)],
                                in_=enew[:, 1, :])
                        if edge_level >= 6 and s == SPB - 1:
                            nc.vector.tensor_copy(agg[:, ds(b * D, D)], scp[:])

                # ---------- node update + allgather ----------
                bounce = dr.tile([cfg.npc, D], F32, tag="bounce", bufs=2)
                h_next = dr.tile([N_PAD, D], F32, tag="hnext", bufs=2)
                for b in range(BPC if "nodeupd" in parts else 0):
                    x = sb.tile([128, D], F32, name="x", tag="x", bufs=2)
                    s1 = sb.tile([128, 1], F32, name="ns1", tag="ns1", bufs=4)
                    nc.vector.scalar_tensor_tensor(
                        out=x[:], in0=hu_sb[b][:], scalar=0.0,
                        in1=agg[:, b * D:(b + 1) * D],
                        op0=ALU.bypass, op1=ALU.add, accum_out=s1[:])
                    sq = sb.tile([128, D], F32, name="nsq", tag="sq", bufs=2)
                    s2 = sb.tile([128, 1], F32, name="ns2", tag="ns2", bufs=4)
                    nc.scalar.activation(sq[:], x[:], AF.Square,
                                         accum_out=s2[:])
                    rstd, nmk = ln_scale_bias(s1, s2, "n", 1)
                    z = sb.tile([128, D], F32, name="z", tag="z", bufs=2)
                    nc.scalar.activation(z[:], x[:], AF.Identity, bias=nmk[:],
                                         scale=rstd[:])
                    y = sb.tile([128, D], F32, name="y", tag="y", bufs=2)
                    nc.vector.tensor_tensor(out=y[:], in0=z[:], in1=nns[:],
                                            op=ALU.mult)
                    y2 = sb.tile([128, D], F32, name="y2", tag="y2", bufs=2)
                    nc.vector.tensor_tensor(out=y2[:], in0=y[:], in1=nnb[:],
                                            op=ALU.add)
                    h_t = sb.tile([128, D], F32, name="h_tn", tag="h_t", bufs=3)
                    nc.sync.dma_start(
                        out=h_t[:],
                        in_=h_prev[ds(core_base + b * 128, 128), :])
                    hnew = sb.tile([128, D], F32, name="hnew", tag="hnew",
                                   bufs=2)
                    nc.vector.scalar_tensor_tensor(
                        out=hnew[:], in0=y2[:], scalar=0.0, in1=h_t[:],
                        op0=ALU.max, op1=ALU.add)
                    nc.sync.dma_start(out=bounce[b * 128:(b + 1) * 128, :],
                                      in_=hnew[:])

                if "coll" in parts:
                    nc.gpsimd.collective_compute(
                        "AllGather", ALU.bypass,
                        replica_groups=[list(range(cfg.n_cores))],
                        ins=[bounce[:]], outs=[h_next[:]])
                    h_prev = h_next
                if "edge" in parts:
                    e_prev = e_next

            if debug_dump:
                nc.sync.dma_start(out=edump_d[:], in_=e_prev[:])
                nc.sync.dma_start(out=hdump_d[:], in_=h_prev[:])
                nc.sync.dma_start(out=adump_d[:], in_=agg[:])

            # ---------- head ----------
            def head_body(bb):
                for s in range(SPB):
                    ebase = bb * C + s * SUPER
                    eT = sb.tile([128, 2, SUPER], F32, name="eTh", tag="eT",
                                 bufs=3)
                    nc.sync.dma_start(out=eT[:, 0, :],
                                      in_=e_prev[0, :, ds(ebase, SUPER)])
                    nc.sync.dma_start(out=eT[:, 1, :],
                                      in_=e_prev[1, :, ds(ebase, SUPER)])
                    ebf = sb.tile([128, 2, SUPER], BF16, name="ebfh", tag="ebf",
                                  bufs=3)
                    nc.vector.tensor_copy(ebf[:], eT[:])
                    h1a = ps.tile([128, SUPER], F32, name="h1a", tag="ps_rv")
                    h1b = ps.tile([128, SUPER], F32, name="h1b", tag="ps_sc",
                                  bufs=2)
                    for j, pj in enumerate((h1a, h1b)):
                        jsl = slice(j * 128, (j + 1) * 128)
                        nc.tensor.matmul(pj[:], lhsT=hw1_sb[:, 0, jsl],
                                         rhs=ebf[:, 0, :], start=True,
                                         stop=False)
                        nc.tensor.matmul(pj[:], lhsT=hw1_sb[:, 1, jsl],
                                         rhs=ebf[:, 1, :], start=False,
                                         stop=True)
                    h1s = sb.tile([128, 2, SUPER], BF16, name="h1s", tag="enew",
                                  bufs=2)
                    for j, pj in enumerate((h1a, h1b)):
                        nc.scalar.activation(h1s[:, j, :], pj[:], AF.Relu,
                                             bias=hb1_sb[:, j:j + 1])
                    op = ps.tile([2, SUPER], F32, name="op", tag="ps_tr")
                    nc.tensor.matmul(op[:], lhsT=hw2_sb[:, 0, :],
                                     rhs=h1s[:, 0, :], start=True, stop=False)
                    nc.tensor.matmul(op[:], lhsT=hw2_sb[:, 1, :],
                                     rhs=h1s[:, 1, :], start=False, stop=True)
                    o_sb = sb.tile([2, SUPER], F32, name="o_sb", tag="o_sb",
                                   bufs=3)
                    nc.scalar.copy(o_sb[:], op[:])
                    nc.sync.dma_start(out=outT_d[:, ds(ebase, SUPER)],
                                      in_=o_sb[:])

            with tc.For_i(0, BPC if "head" in parts else 0, 1,
                          name="head") as bb:
                head_body(bb)

    nc.compile()
    return nc


_CACHE = {}


def get_program(cfg: Cfg):
    if cfg not in _CACHE:
        _CACHE[cfg] = build_program(cfg)
    return _CACHE[cfg]


def make_in_maps(per_core, h0, w, cfg):
    in_maps = []
    for c in range(cfg.n_cores):
        pc = per_core[c]
        in_maps.append({
            "e0T": pc["e0T"], "h0": h0,
            "srel": pc["srel"], "gq16": pc["gq16"], "gd16": pc["gd16"],
            "Pw": w["Pw"], "Qw": w["Qw"], "Uw": w["Uw"], "RVw": w["RVw"],
            "W1": w["W1"], "W2": w["W2"], "b1": w["b1"], "b2": w["b2"],
            "nns": w["nns"], "nnb": w["nnb"],
            "HW1": w["HW1"], "hb1": w["hb1"], "HW2": w["HW2"],
            "iota": w["iota"],
        })
    return in_maps


def assemble_output(results, per_core, params, cfg):
    out = np.zeros((cfg.n_edges, 2), np.float32)
    for c in range(cfg.n_cores):
        outT = results[c]["outT"]
        pc = per_core[c]
        out[pc["ids"]] = outT[:, pc["slot"]].T
    return out + np.asarray(params["head_b2"], np.float32)[None, :]


def kernel(node_coords, edge_index, edge_distances, x_t, t, params):
    node_coords = np.asarray(node_coords)
    params = {k: np.asarray(v) for k, v in params.items()}

    h0, per_core, w, cfg = host_prepare(
        node_coords, edge_index, np.asarray(edge_distances), np.asarray(x_t),
        np.asarray(t), params)
    nc = get_program(cfg)
    in_maps = make_in_maps(per_core, h0, w, cfg)
    res = bass_utils.run_bass_kernel_spmd(
        nc, in_maps, core_ids=list(range(cfg.n_cores)))
    return assemble_output(res.results, per_core, params, cfg)
